# revision 21
# baseline (speedup 1.0000x reference)
"""Trainium2 Bass kernel for nn_Attention2 (dense transformer block with
softmax over the heads axis).

Computation per (n, t) batch b (B = n*t = 4096 total, X_b = x[n,:,t,:].T is
[vv=25, c=512]):
    qkv = X_b @ w_qkv.T, split into q,k,v heads [h=8, 25, hd=64]
    s[h,i,j] = (q[h,i,:] . k[h,j,:]) / 8      (scale folded into w_q on host)
    p = softmax over h (axis 0)
    o[h,i,:] = sum_j p[h,i,j] v[h,j,:]  -> [25, 512] -> @ w_proj.T
    out[n,:,t,:] = result.T

Sharding: data-parallel over n, 2 n-values (512 batches) per core, 8 cores.

v4: baseline per-sub attention core (identical tiles/ops/dtypes), emitted in
a software-pipelined order: iteration g emits
    [scores g-1] [qk GEMMs g] [softmax g-1] [v GEMMs g] [out g-1] [proj g-1]
so the PE always has independent big-GEMM work covering the softmax DVE/ACT
latency (no head-of-line stalls, HAM stays warm).
"""
import os
import numpy as np
import concourse.bass as bass
import concourse.mybir as mybir
import concourse.tile as tile
from concourse.bass_utils import run_bass_kernel_spmd
from concourse.vector_clock import ScopedClock, VectorClock

F32 = mybir.dt.float32
F16 = mybir.dt.float16

N_CORES = 8
NN_PER_CORE = 2        # n values per core
T = 256
VV = 25
C = 512
H = 8
HD = 64
TG = 16                # t values (batches) per group
NGROUPS = NN_PER_CORE * (T // TG)   # 32 groups per core
NB = TG * VV           # 400 moving columns per group

SIM_MEMSET = bool(int(os.environ.get("SIM_MEMSET", "0")))


def _split_drain_and_barrier(self, tick_clock, wait_clock):
    # walrus caps sync-wait commands at 1 for CTRL_NO; split the kernel-tail
    # drain into one drain per pending proc.
    vc = tick_clock.global_clock
    n = len(vc)
    for i in range(n):
        if vc[i] == 0:
            continue
        sub = VectorClock([vc[j] if j == i else 0 for j in range(n)])
        d = self.nc.sync.drain()
        wait_clock.add_sem_waits(d.ins, ScopedClock({None: sub}))
    self.nc.all_engine_barrier()
    assert self.sems is not None
    popped = self.nc._tile_sem_poison_stack.pop()
    assert popped is self._sem_poison
    self.nc.clear_and_free_semaphores(list(self.sems.allocated().values()))
    self.nc.all_engine_barrier()


tile.TileContext._drain_and_barrier = _split_drain_and_barrier


def split_excess_waits(nc, limit=1):
    """walrus codegen allows very few sync-wait commands per instruction
    (1 for matmul/drain/DMA structs).  Move excess waits onto same-engine
    NoOp carriers inserted just before the instruction — same semantics,
    since each engine executes its queue in order."""
    k = 0
    for fn in nc.m.functions:
        for bb in fn.blocks:
            out = []
            for ins in bb.instructions:
                si = ins.sync_info
                waits = list(si.on_wait) if si is not None and si.on_wait else []
                if len(waits) > limit:
                    keep = waits[-limit:]
                    for w in waits[:-limit]:
                        nop = mybir.InstNoOp(
                            name=f"WC-{k}", ins=[], outs=[], engine=ins.engine
                        )
                        k += 1
                        nop.sync_info = mybir.SyncInfo(on_wait=[w], on_update=[])
                        out.append(nop)
                    si.on_wait = keep
                out.append(ins)
            bb.instructions[:] = out
    return k


def build_nc():
    nc = bass.Bass()
    X = nc.declare_dram_parameter("x", [NN_PER_CORE, C, T, VV], F16, isOutput=False)
    WQK = nc.declare_dram_parameter("wqkT", [C, 2 * C], F16, isOutput=False)
    WV = nc.declare_dram_parameter("wvT", [C, C], F16, isOutput=False)
    WP = nc.declare_dram_parameter("wprojT", [C, C], F16, isOutput=False)
    Y = nc.declare_dram_parameter("y", [NN_PER_CORE, C, T, VV], F32, isOutput=True)

    with tile.TileContext(nc) as tc:
        with (
            tc.tile_pool(name="consts", bufs=1) as consts,
            tc.tile_pool(name="pg", bufs=2) as pg,
            tc.tile_pool(name="pbig", bufs=3, space="PSUM") as pbig,
            tc.tile_pool(name="psmall", bufs=1, space="PSUM") as psmall,
        ):
            # ---- load weights (fp16) ----
            wqk_r, wv_r, wp_r = [], [], []
            for kc in range(4):
                r0 = consts.tile([128, 2 * C], F16, tag=f"wqkr{kc}")
                nc.sync.dma_start(out=r0, in_=WQK[kc * 128:(kc + 1) * 128, :])
                wqk_r.append(r0)
                r1 = consts.tile([128, C], F16, tag=f"wvr{kc}")
                nc.sync.dma_start(out=r1, in_=WV[kc * 128:(kc + 1) * 128, :])
                wv_r.append(r1)
                r2 = consts.tile([128, C], F16, tag=f"wpr{kc}")
                nc.sync.dma_start(out=r2, in_=WP[kc * 128:(kc + 1) * 128, :])
                wp_r.append(r2)

            prev = None

            for g in range(NGROUPS + 1):
                cur = None
                if g < NGROUPS:
                    nn = g // (T // TG)
                    t0 = (g % (T // TG)) * TG
                    xp = []
                    for kc in range(4):
                        xq = pg.tile([128, TG, 32], F16, tag=f"xp{kc}")
                        if SIM_MEMSET:
                            nc.vector.memset(xq[:, :, VV:32], 0.0)
                        nc.sync.dma_start(
                            out=xq[:, :, 0:VV],
                            in_=X[nn, kc * 128:(kc + 1) * 128, t0:t0 + TG, :],
                        )
                        xp.append(xq)
                    cur = {"xp": xp, "nn": nn, "t0": t0}

                # ---- scores g-1 (sub pairs) interleaved with qk GEMM halves g:
                # pair-merged psm (one bank per par, 2 sub slots) frees 2 PSUM
                # banks for po double-buffering; the qk chains between pairs
                # cover the exp latency so pair 1 never stalls the PE ----
                def emit_scores_pair(prevd, sp):
                    qkT = prevd["qkT"]
                    psmp = [
                        psmall.tile([128, 2, 4, VV], F32, tag=f"psm{par}",
                                    name=f"psm{par}", bufs=1)
                        for par in range(2)
                    ]
                    if SIM_MEMSET:
                        for par in range(2):
                            nc.vector.memset(psmp[par][:], 0.0)
                    for ss in range(2):
                        sub = sp * 2 + ss
                        bcol0 = sub * 4 * VV
                        for h in range(H):
                            m, par, r0 = h // 2, h % 2, (h % 2) * 64
                            for b4 in range(4):
                                bcol = bcol0 + b4 * VV
                                nc.tensor.matmul(
                                    psmp[par][b4 * 32:b4 * 32 + 25, ss, m, :],
                                    qkT[4 + m][r0:r0 + 64, bcol:bcol + VV],
                                    qkT[m][r0:r0 + 64, bcol:bcol + VV],
                                    start=True, stop=True,
                                    skip_group_check=True,
                                    tile_position=(r0, b4 * 32),
                                )
                        e_t = pg.tile([128, VV, H], F32, tag="e_t", bufs=5)
                        for par in range(2):
                            nc.scalar.activation(
                                e_t[:, :, par::2],
                                psmp[par][:, ss, :, :].rearrange("p m i -> p i m"),
                                mybir.ActivationFunctionType.Exp,
                            )
                        prevd["e_ts"].append(e_t)

                def emit_qk_half(curd, mlo, mhi):
                    xp = curd["xp"]
                    for m in range(mlo, mhi):
                        pq = pbig.tile([128, NB], F32, tag="big")
                        for kc in range(4):
                            nc.tensor.matmul(
                                pq[:],
                                wqk_r[kc][:, m * 128:(m + 1) * 128],
                                xp[kc][:, :, 0:VV],
                                start=(kc == 0), stop=(kc == 3),
                            )
                        qc = pg.tile([128, NB], F16, tag=f"qkT{m}")
                        nc.vector.tensor_copy(qc[:], pq[:])
                        curd["qkT"].append(qc)

                if prev is not None:
                    prev["e_ts"] = []
                if cur is not None:
                    cur["qkT"] = []
                if prev is not None:
                    emit_scores_pair(prev, 0)
                if cur is not None:
                    emit_qk_half(cur, 0, 4)
                if prev is not None:
                    emit_scores_pair(prev, 1)
                if cur is not None:
                    emit_qk_half(cur, 4, 8)

                # ---- softmax chains for group g-1 (per sub, baseline ops) ----
                if prev is not None:
                    p2s = []
                    for sub in range(4):
                        e_t = prev["e_ts"][sub]
                        D = pg.tile([128, VV], F32, tag="D", bufs=3)
                        nc.vector.reduce_sum(out=D[:], in_=e_t[:],
                                             axis=mybir.AxisListType.X)
                        rD = pg.tile([128, VV], F32, tag="rD", bufs=3)
                        nc.vector.reciprocal(rD[:], D[:])
                        p2 = [pg.tile([64, VV, H], F16, tag=f"p2{sub}{q}",
                                      name=f"p2{q}") for q in range(2)]
                        for q in range(2):
                            nc.vector.tensor_mul(
                                p2[q][:],
                                e_t[q * 64:(q + 1) * 64, :, :],
                                rD[q * 64:(q + 1) * 64, :]
                                .unsqueeze(2).broadcast_to([64, VV, H]),
                            )
                        p2s.append(p2)
                    prev["p2s"] = p2s

                # ---- v GEMMs for group g (per sub, baseline tiles) ----
                if cur is not None:
                    xp = cur["xp"]
                    v2s = []
                    for sub in range(4):
                        pv = pbig.tile([128, C], F32, tag="big", name="pv")
                        for kc in range(4):
                            nc.tensor.matmul(
                                pv[:],
                                xp[kc][:, sub * 4:sub * 4 + 4, :],
                                wv_r[kc][:],
                                start=(kc == 0), stop=(kc == 3),
                            )
                        v2 = [pg.tile([64, C], F16, tag=f"v2{sub}{q}",
                                      name=f"v2{q}") for q in range(2)]
                        for q in range(2):
                            nc.scalar.activation(
                                v2[q][:, :], pv[q * 64:(q + 1) * 64, :],
                                mybir.ActivationFunctionType.Copy,
                            )
                        v2s.append(v2)
                    cur["v2s"] = v2s

                # ---- out-matmuls + proj for group g-1 (baseline tiles) ----
                if prev is not None:
                    oT = pg.tile([128, 4, NB], F16, tag="oT", name="oT")
                    for sub in range(4):
                        v2 = prev["v2s"][sub]
                        p2 = prev["p2s"][sub]
                        po = [
                            psmall.tile([128, 4, 2 * VV], F32, tag=f"po{e}",
                                        name=f"po{e}", bufs=1 + e)
                            for e in range(2)
                        ]
                        for b4 in range(4):
                            q, e = b4 // 2, b4 % 2
                            for h in range(H):
                                m, c0 = h // 2, (h % 2) * 64
                                nc.tensor.matmul(
                                    po[e][c0:c0 + 64, m, q * VV:(q + 1) * VV],
                                    v2[q][e * 32:e * 32 + 25, h * HD:(h + 1) * HD],
                                    p2[q][e * 32:e * 32 + 25, :, h],
                                    start=True, stop=True,
                                    skip_group_check=True,
                                    tile_position=(e * 32, c0),
                                )
                        for e in range(2):
                            dst = oT[:].rearrange(
                                "p m (b i) -> p m b i", i=VV
                            )[:, :, sub * 4 + e:sub * 4 + e + 3:2, :]
                            nc.vector.tensor_copy(
                                dst, po[e][:].rearrange(
                                    "p m (b i) -> p m b i", i=VV
                                )
                            )

                    pnn, pt0 = prev["nn"], prev["t0"]
                    for co in range(4):
                        pf = pbig.tile([128, NB], F32, tag="big")
                        for kc in range(4):
                            nc.tensor.matmul(
                                pf[:],
                                wp_r[kc][:, co * 128:(co + 1) * 128],
                                oT[:, kc, :],
                                start=(kc == 0), stop=(kc == 3),
                            )
                        fin = pg.tile([128, NB], F32, tag=f"fin{co}")
                        if co % 2 == 0:
                            nc.vector.tensor_copy(fin[:], pf[:])
                        else:
                            nc.scalar.activation(
                                fin[:], pf[:], mybir.ActivationFunctionType.Copy,
                            )
                        nc.sync.dma_start(
                            out=Y[pnn, co * 128:(co + 1) * 128, pt0:pt0 + TG, :],
                            in_=fin[:].rearrange("p (t v) -> p t v", t=TG),
                        )

                prev = cur
    return nc


LAST_RESULT = {}


def kernel(x: np.ndarray, w_qkv: np.ndarray, w_proj: np.ndarray,
           _trace: bool = False) -> np.ndarray:
    n, c, t, vv = x.shape
    assert (n, c, t, vv) == (16, 512, 256, 25)
    scale = np.float32((c // H) ** -0.5)

    wq = w_qkv[:c] * scale
    wk = w_qkv[c:2 * c]
    wv = w_qkv[2 * c:]
    wqkT = np.ascontiguousarray(np.concatenate([wq, wk], axis=0).T.astype(np.float16))
    wvT = np.ascontiguousarray(wv.T.astype(np.float16))
    wprojT = np.ascontiguousarray(w_proj.T.astype(np.float16))

    nc = build_nc()
    split_excess_waits(nc)
    in_maps = []
    for core in range(N_CORES):
        shard = np.ascontiguousarray(
            x[core * NN_PER_CORE:(core + 1) * NN_PER_CORE].astype(np.float16)
        )
        in_maps.append({"x": shard, "wqkT": wqkT, "wvT": wvT, "wprojT": wprojT})

    kw = {}
    if _trace:
        import tempfile
        kw = dict(trace=True, tmpdir=tempfile.mkdtemp(prefix="attn2_trace_"))
    res = run_bass_kernel_spmd(nc, in_maps, list(range(N_CORES)), **kw)
    LAST_RESULT["res"] = res
    LAST_RESULT["tmpdir"] = kw.get("tmpdir")
    out = np.empty((n, c, t, vv), dtype=np.float32)
    for core in range(N_CORES):
        out[core * NN_PER_CORE:(core + 1) * NN_PER_CORE] = res.results[core]["y"]
    return out


# revision 22
# speedup vs baseline: 1.1836x; 1.1836x over previous
"""Trainium2 Bass kernel for nn_Attention2 (dense transformer block with
softmax over the heads axis).

Computation per (n, t) batch b (B = n*t = 4096 total, X_b = x[n,:,t,:].T is
[vv=25, c=512]):
    qkv = X_b @ w_qkv.T, split into q,k,v heads [h=8, 25, hd=64]
    s[h,i,j] = (q[h,i,:] . k[h,j,:]) / 8      (scale folded into w_q on host)
    p = softmax over h (axis 0)
    o[h,i,:] = sum_j p[h,i,j] v[h,j,:]  -> [25, 512] -> @ w_proj.T
    out[n,:,t,:] = result.T

Sharding: data-parallel over n, 2 n-values (512 batches) per core, 8 cores.

v4: baseline per-sub attention core (identical tiles/ops/dtypes), emitted in
a software-pipelined order: iteration g emits
    [scores g-1] [qk GEMMs g] [softmax g-1] [v GEMMs g] [out g-1] [proj g-1]
so the PE always has independent big-GEMM work covering the softmax DVE/ACT
latency (no head-of-line stalls, HAM stays warm).
"""
import os
import numpy as np
import concourse.bass as bass
import concourse.mybir as mybir
import concourse.tile as tile
from concourse.bass_utils import run_bass_kernel_spmd
from concourse.vector_clock import ScopedClock, VectorClock

F32 = mybir.dt.float32
F16 = mybir.dt.float16

N_CORES = 8
NN_PER_CORE = 2        # n values per core
T = 256
VV = 25
C = 512
H = 8
HD = 64
TG = 16                # t values (batches) per group
NGROUPS = NN_PER_CORE * (T // TG)   # 32 groups per core
NB = TG * VV           # 400 moving columns per group

SIM_MEMSET = bool(int(os.environ.get("SIM_MEMSET", "0")))


def _split_drain_and_barrier(self, tick_clock, wait_clock):
    # walrus caps sync-wait commands at 1 for CTRL_NO; split the kernel-tail
    # drain into one drain per pending proc.
    vc = tick_clock.global_clock
    n = len(vc)
    for i in range(n):
        if vc[i] == 0:
            continue
        sub = VectorClock([vc[j] if j == i else 0 for j in range(n)])
        d = self.nc.sync.drain()
        wait_clock.add_sem_waits(d.ins, ScopedClock({None: sub}))
    self.nc.all_engine_barrier()
    assert self.sems is not None
    popped = self.nc._tile_sem_poison_stack.pop()
    assert popped is self._sem_poison
    self.nc.clear_and_free_semaphores(list(self.sems.allocated().values()))
    self.nc.all_engine_barrier()


tile.TileContext._drain_and_barrier = _split_drain_and_barrier


def split_excess_waits(nc, limit=1):
    """walrus codegen allows very few sync-wait commands per instruction
    (1 for matmul/drain/DMA structs).  Move excess waits onto same-engine
    NoOp carriers inserted just before the instruction — same semantics,
    since each engine executes its queue in order."""
    k = 0
    for fn in nc.m.functions:
        for bb in fn.blocks:
            out = []
            for ins in bb.instructions:
                si = ins.sync_info
                waits = list(si.on_wait) if si is not None and si.on_wait else []
                if len(waits) > limit:
                    keep = waits[-limit:]
                    for w in waits[:-limit]:
                        nop = mybir.InstNoOp(
                            name=f"WC-{k}", ins=[], outs=[], engine=ins.engine
                        )
                        k += 1
                        nop.sync_info = mybir.SyncInfo(on_wait=[w], on_update=[])
                        out.append(nop)
                    si.on_wait = keep
                out.append(ins)
            bb.instructions[:] = out
    return k


def build_nc():
    nc = bass.Bass()
    X = nc.declare_dram_parameter("x", [NN_PER_CORE, C, T, VV], F16, isOutput=False)
    WQK = nc.declare_dram_parameter("wqkT", [C, 2 * C], F16, isOutput=False)
    WV = nc.declare_dram_parameter("wvT", [C, C], F16, isOutput=False)
    WP = nc.declare_dram_parameter("wprojT", [C, C], F16, isOutput=False)
    Y = nc.declare_dram_parameter("y", [NN_PER_CORE, C, T, VV], F32, isOutput=True)

    with tile.TileContext(nc) as tc:
        with (
            tc.tile_pool(name="consts", bufs=1) as consts,
            tc.tile_pool(name="pg", bufs=2) as pg,
            tc.tile_pool(name="pbig", bufs=2, space="PSUM") as pbig,
            tc.tile_pool(name="psmall", bufs=1, space="PSUM") as psmall,
        ):
            # ---- load weights (fp16) ----
            wqk_r, wv_r, wp_r = [], [], []
            for kc in range(4):
                r0 = consts.tile([128, 2 * C], F16, tag=f"wqkr{kc}")
                nc.sync.dma_start(out=r0, in_=WQK[kc * 128:(kc + 1) * 128, :])
                wqk_r.append(r0)
                r1 = consts.tile([128, C], F16, tag=f"wvr{kc}")
                nc.sync.dma_start(out=r1, in_=WV[kc * 128:(kc + 1) * 128, :])
                wv_r.append(r1)
                r2 = consts.tile([128, C], F16, tag=f"wpr{kc}")
                nc.sync.dma_start(out=r2, in_=WP[kc * 128:(kc + 1) * 128, :])
                wp_r.append(r2)

            prev = None

            for g in range(NGROUPS + 1):
                cur = None
                if g < NGROUPS:
                    nn = g // (T // TG)
                    t0 = (g % (T // TG)) * TG
                    xp = []
                    for kc in range(4):
                        xq = pg.tile([128, TG, 32], F16, tag=f"xp{kc}")
                        if SIM_MEMSET:
                            nc.vector.memset(xq[:, :, VV:32], 0.0)
                        nc.sync.dma_start(
                            out=xq[:, :, 0:VV],
                            in_=X[nn, kc * 128:(kc + 1) * 128, t0:t0 + TG, :],
                        )
                        xp.append(xq)
                    cur = {"xp": xp, "nn": nn, "t0": t0}

                # ---- scores g-1 (sub pairs) interleaved with qk GEMM halves g:
                # pair-merged psm (one bank per par, 2 sub slots) frees 2 PSUM
                # banks for po double-buffering; the qk chains between pairs
                # cover the exp latency so pair 1 never stalls the PE ----
                def emit_scores_pair(prevd, sp):
                    qkT = prevd["qkT"]
                    psmp = [
                        psmall.tile([128, 2, 4, VV], F32, tag=f"psm{par}",
                                    name=f"psm{par}", bufs=1)
                        for par in range(2)
                    ]
                    if SIM_MEMSET:
                        for par in range(2):
                            nc.vector.memset(psmp[par][:], 0.0)
                    for ss in range(2):
                        sub = sp * 2 + ss
                        bcol0 = sub * 4 * VV
                        for h in range(H):
                            m, par, r0 = h // 2, h % 2, (h % 2) * 64
                            for b4 in range(4):
                                bcol = bcol0 + b4 * VV
                                nc.tensor.matmul(
                                    psmp[par][b4 * 32:b4 * 32 + 25, ss, m, :],
                                    qkT[4 + m][r0:r0 + 64, bcol:bcol + VV],
                                    qkT[m][r0:r0 + 64, bcol:bcol + VV],
                                    start=True, stop=True,
                                    skip_group_check=True,
                                    tile_position=(r0, b4 * 32),
                                )
                        e_t = pg.tile([128, VV, H], F32, tag="e_t", bufs=5)
                        for par in range(2):
                            nc.scalar.activation(
                                e_t[:, :, par::2],
                                psmp[par][:, ss, :, :].rearrange("p m i -> p i m"),
                                mybir.ActivationFunctionType.Exp,
                            )
                        prevd["e_ts"].append(e_t)

                def emit_qk_half(curd, mlo, mhi):
                    xp = curd["xp"]
                    for m in range(mlo, mhi):
                        pq = pbig.tile([128, NB], F32, tag="big")
                        for kc in range(4):
                            nc.tensor.matmul(
                                pq[:],
                                wqk_r[kc][:, m * 128:(m + 1) * 128],
                                xp[kc][:, :, 0:VV],
                                start=(kc == 0), stop=(kc == 3),
                            )
                        qc = pg.tile([128, NB], F16, tag=f"qkT{m}")
                        nc.vector.tensor_copy(qc[:], pq[:])
                        curd["qkT"].append(qc)

                if prev is not None:
                    prev["e_ts"] = []
                if cur is not None:
                    cur["qkT"] = []
                if prev is not None:
                    emit_scores_pair(prev, 0)
                if cur is not None:
                    emit_qk_half(cur, 0, 4)
                if prev is not None:
                    emit_scores_pair(prev, 1)
                if cur is not None:
                    emit_qk_half(cur, 4, 8)

                # ---- softmax chains for group g-1 (per sub, baseline ops) ----
                if prev is not None:
                    p2s = []
                    for sub in range(4):
                        e_t = prev["e_ts"][sub]
                        D = pg.tile([128, VV], F32, tag="D", bufs=3)
                        nc.vector.reduce_sum(out=D[:], in_=e_t[:],
                                             axis=mybir.AxisListType.X)
                        rD = pg.tile([128, VV], F32, tag="rD", bufs=3)
                        nc.vector.reciprocal(rD[:], D[:])
                        p2 = [pg.tile([64, VV, H], F16, tag=f"p2{sub}{q}",
                                      name=f"p2{q}") for q in range(2)]
                        for q in range(2):
                            nc.vector.tensor_mul(
                                p2[q][:],
                                e_t[q * 64:(q + 1) * 64, :, :],
                                rD[q * 64:(q + 1) * 64, :]
                                .unsqueeze(2).broadcast_to([64, VV, H]),
                            )
                        p2s.append(p2)
                    prev["p2s"] = p2s

                # ---- v GEMMs for group g (per sub, baseline tiles) ----
                if cur is not None:
                    xp = cur["xp"]
                    v2s = []
                    for sub in range(4):
                        pv = pbig.tile([128, C], F32, tag="big", name="pv")
                        for kc in range(4):
                            nc.tensor.matmul(
                                pv[:],
                                xp[kc][:, sub * 4:sub * 4 + 4, :],
                                wv_r[kc][:],
                                start=(kc == 0), stop=(kc == 3),
                            )
                        v2 = [pg.tile([64, C], F16, tag=f"v2{sub}{q}",
                                      name=f"v2{q}") for q in range(2)]
                        for q in range(2):
                            nc.scalar.activation(
                                v2[q][:, :], pv[q * 64:(q + 1) * 64, :],
                                mybir.ActivationFunctionType.Copy,
                            )
                        v2s.append(v2)
                    cur["v2s"] = v2s

                # ---- out-matmuls + proj for group g-1 (baseline tiles) ----
                if prev is not None:
                    oT = pg.tile([128, 4, NB], F16, tag="oT", name="oT")
                    for sub in range(4):
                        v2 = prev["v2s"][sub]
                        p2 = prev["p2s"][sub]
                        po = [
                            psmall.tile([128, 4, 2 * VV], F32, tag=f"po{e}",
                                        name=f"po{e}", bufs=2)
                            for e in range(2)
                        ]
                        for b4 in range(4):
                            q, e = b4 // 2, b4 % 2
                            for h in range(H):
                                m, c0 = h // 2, (h % 2) * 64
                                nc.tensor.matmul(
                                    po[e][c0:c0 + 64, m, q * VV:(q + 1) * VV],
                                    v2[q][e * 32:e * 32 + 25, h * HD:(h + 1) * HD],
                                    p2[q][e * 32:e * 32 + 25, :, h],
                                    start=True, stop=True,
                                    skip_group_check=True,
                                    tile_position=(e * 32, c0),
                                )
                        for e in range(2):
                            dst = oT[:].rearrange(
                                "p m (b i) -> p m b i", i=VV
                            )[:, :, sub * 4 + e:sub * 4 + e + 3:2, :]
                            nc.vector.tensor_copy(
                                dst, po[e][:].rearrange(
                                    "p m (b i) -> p m b i", i=VV
                                )
                            )

                    pnn, pt0 = prev["nn"], prev["t0"]
                    for co in range(4):
                        pf = pbig.tile([128, NB], F32, tag="big")
                        for kc in range(4):
                            nc.tensor.matmul(
                                pf[:],
                                wp_r[kc][:, co * 128:(co + 1) * 128],
                                oT[:, kc, :],
                                start=(kc == 0), stop=(kc == 3),
                            )
                        fin = pg.tile([128, NB], F32, tag=f"fin{co}")
                        if co % 2 == 0:
                            nc.vector.tensor_copy(fin[:], pf[:])
                        else:
                            nc.scalar.activation(
                                fin[:], pf[:], mybir.ActivationFunctionType.Copy,
                            )
                        nc.sync.dma_start(
                            out=Y[pnn, co * 128:(co + 1) * 128, pt0:pt0 + TG, :],
                            in_=fin[:].rearrange("p (t v) -> p t v", t=TG),
                        )

                prev = cur
    return nc


LAST_RESULT = {}


def kernel(x: np.ndarray, w_qkv: np.ndarray, w_proj: np.ndarray,
           _trace: bool = False) -> np.ndarray:
    n, c, t, vv = x.shape
    assert (n, c, t, vv) == (16, 512, 256, 25)
    scale = np.float32((c // H) ** -0.5)

    wq = w_qkv[:c] * scale
    wk = w_qkv[c:2 * c]
    wv = w_qkv[2 * c:]
    wqkT = np.ascontiguousarray(np.concatenate([wq, wk], axis=0).T.astype(np.float16))
    wvT = np.ascontiguousarray(wv.T.astype(np.float16))
    wprojT = np.ascontiguousarray(w_proj.T.astype(np.float16))

    nc = build_nc()
    split_excess_waits(nc)
    in_maps = []
    for core in range(N_CORES):
        shard = np.ascontiguousarray(
            x[core * NN_PER_CORE:(core + 1) * NN_PER_CORE].astype(np.float16)
        )
        in_maps.append({"x": shard, "wqkT": wqkT, "wvT": wvT, "wprojT": wprojT})

    kw = {}
    if _trace:
        import tempfile
        kw = dict(trace=True, tmpdir=tempfile.mkdtemp(prefix="attn2_trace_"))
    res = run_bass_kernel_spmd(nc, in_maps, list(range(N_CORES)), **kw)
    LAST_RESULT["res"] = res
    LAST_RESULT["tmpdir"] = kw.get("tmpdir")
    out = np.empty((n, c, t, vv), dtype=np.float32)
    for core in range(N_CORES):
        out[core * NN_PER_CORE:(core + 1) * NN_PER_CORE] = res.results[core]["y"]
    return out
